# revision 9
# baseline (speedup 1.0000x reference)
"""GAU (gated attention unit) Trainium2 kernel — fp8 DoubleRow version.

Problem: B=8, S=2048, D=512, QK=128, HID=1024 (expansion 2x), fp32 I/O.
Sharding: pure data-parallel - one batch element per NeuronCore (8 cores).

Per-core pipeline (token tiles of 128; heavy matmuls in fp8-e4m3 with
MatmulPerfMode.DoubleRow pairing two K=128 slices per pass; fp32 PSUM):
  P1  LN stats via DVE bn_stats/bn_aggr; normalize on ACT
      (Identity(x*rstd - mu*rstd)) -> bf16; PE-transpose 128x128 blocks;
      ACT copies PSUM-bf16 -> SBUF fp8 normedT [128, KD+1, S].
      normedT slice KD is a constant ones-row (partition 0) so the v
      projection picks up its bias as a 5th contraction tile.
  P2a ZT = Wqk.T @ normedT (fp8 DR), silu+bias+descale on ACT -> zt f32;
      qT/kT = zt*gamma' + beta' on DVE -> bf16 (scaled by a, b/S).
  P2b v = silu(normed @ Whv + bhv) token-major: fp8 DR pairs + 5th bias
      tile; silu+descale on ACT straight from PSUM -> vtok fp8.
  P3 per 512-query chunk:
      gate   = silu(Whg.T @ normedT * 1/s_wh + bhg)  (fp8 DR; ACT) f32
      sim'   = kT_tile.T @ qT_chunk (bf16 PE)   [s_sim-scaled]
      at     = relu(sim')^2: ACT Relu (PSUM->SBUF f32), DVE square -> fp8
      VT     = sum_kt at.T-pairs (fp8 DR, 8 pairs)  [128 hid, 512 q]
      vtg    = (VT * s_vtg) * gate on DVE -> fp8
      out    = token-major: per 128-token tile, PSUM accumulates
               idr @ x (fp32r inject of residual, scaled 1/s_fin) plus
               vtg-pairs.T @ Wo' (fp8 DR);
               final: DVE stt (psum * s_fin + bo_row) -> DMA out.

Scales are host-calibrated per input set (64-token sample) as powers of
two so fp8 ranges stay safe for arbitrary input magnitudes.

Hardware facts this kernel relies on (probed on trn2/axon):
  - DVE/GPSIMD must never READ bf16/fp8 tiles; DVE may WRITE fp8.
  - DVE ops may read at most ONE operand from PSUM.
  - fp8 PE transpose needs stride-2 out; transpose bf16, convert in the
    ACT PSUM->SBUF copy instead.
  - DoubleRow needs both operands fp8 with 3D APs [128, 2, N].
  - Every declared ExternalInput must be consumed.
"""

import os
import sys

import numpy as np

for _p in ("/opt/trn_rl_repo", "/root/.axon_site/_ro/trn_rl_repo"):
    if os.path.isdir(_p) and _p not in sys.path:
        sys.path.insert(0, _p)

import ml_dtypes  # noqa: E402
import concourse.bass as bass  # noqa: E402
import concourse.tile as tile  # noqa: E402
from concourse import bacc, mybir  # noqa: E402
from concourse.bass_utils import run_bass_kernel_spmd  # noqa: E402

B, S, D = 8, 2048, 512
QK = 128
HID = 1024
EPS = 1e-5
NCORES = 8

TT = S // 128     # 16 token tiles
KD = D // 128     # 4 k-tiles over D
MH = HID // 128   # 8 hid slices
NQ = 4            # 4 query superchunks of 512

dt = mybir.dt
AF = mybir.ActivationFunctionType
ALU = mybir.AluOpType
DR = mybir.MatmulPerfMode.DoubleRow
BF16 = ml_dtypes.bfloat16
F8 = ml_dtypes.float8_e4m3

# how many of the 64 relu^2 tiles per iter use ACT Square instead of DVE
N_ACT_SQ = int(os.environ.get("KACTSQ", "0"))

_COMPILED = {}


def _build(loops: int = 1):
    nc = bacc.Bacc("TRN2", target_bir_lowering=False, debug=False,
                   num_devices=NCORES)
    f8 = dt.float8e4
    aps = {
        "x": nc.dram_tensor("x", [S, D], dt.float32, kind="ExternalInput").ap(),
        "whv": nc.dram_tensor("whv", [128, KD + 1, HID], f8, kind="ExternalInput").ap(),
        "whg": nc.dram_tensor("whg", [128, KD, HID], f8, kind="ExternalInput").ap(),
        "wqk": nc.dram_tensor("wqk", [128, KD, QK], f8, kind="ExternalInput").ap(),
        "wo": nc.dram_tensor("wo", [128, MH, D], f8, kind="ExternalInput").ap(),
        "bhg": nc.dram_tensor("bhg", [128, MH], dt.float32, kind="ExternalInput").ap(),
        "bqk": nc.dram_tensor("bqk", [128, 1], dt.float32, kind="ExternalInput").ap(),
        "gq": nc.dram_tensor("gq", [128, 1], dt.float32, kind="ExternalInput").ap(),
        "bq": nc.dram_tensor("bq", [128, 1], dt.float32, kind="ExternalInput").ap(),
        "gk": nc.dram_tensor("gk", [128, 1], dt.float32, kind="ExternalInput").ap(),
        "bk": nc.dram_tensor("bk", [128, 1], dt.float32, kind="ExternalInput").ap(),
        "bo_row": nc.dram_tensor("bo_row", [128, D], dt.float32, kind="ExternalInput").ap(),
        "idb": nc.dram_tensor("idb", [128, 128], dt.bfloat16, kind="ExternalInput").ap(),
    }
    out_ap = nc.dram_tensor("out", [S, D], dt.float32, kind="ExternalOutput").ap()
    with tile.TileContext(nc) as tc:
        _emit(nc, tc, loops, aps, out_ap)
    nc.compile()
    return nc


def _emit(nc, tc, loops, aps, ap_out):
    from contextlib import ExitStack

    f8 = dt.float8e4
    ap_x = aps["x"]
    ctx = ExitStack()
    with ctx:
        cst = ctx.enter_context(tc.tile_pool(name="cst", bufs=1))
        wpool = ctx.enter_context(tc.tile_pool(name="wpool", bufs=1))
        res = ctx.enter_context(tc.tile_pool(name="res", bufs=1))

        idb = cst.tile([128, 128], dt.bfloat16, name="idb")
        nc.sync.dma_start(idb[:], aps["idb"][:])
        eps_t = cst.tile([128, 1], dt.float32, name="eps_t")
        nc.vector.memset(eps_t[:], EPS)

        vecs = {}
        for nm, width in (("bhg", MH), ("bqk", 1), ("gq", 1), ("bq", 1),
                          ("gk", 1), ("bk", 1), ("bo_row", D)):
            vecs[nm] = cst.tile([128, width], dt.float32, name=f"{nm}_t")
            nc.sync.dma_start(vecs[nm][:], aps[nm][:])

        whv = wpool.tile([128, KD + 1, HID], f8, name="whv")
        nc.sync.dma_start(whv[:], aps["whv"][:])
        whg = wpool.tile([128, KD, HID], f8, name="whg")
        nc.sync.dma_start(whg[:], aps["whg"][:])
        wqk = wpool.tile([128, KD, QK], f8, name="wqk")
        nc.sync.dma_start(wqk[:], aps["wqk"][:])
        wo = wpool.tile([128, MH, D], f8, name="wo")
        nc.sync.dma_start(wo[:], aps["wo"][:])

        xt = res.tile([128, TT, D], dt.float32, name="xt")
        normedT = res.tile([128, KD + 1, S], f8, name="normedT")
        vtok = res.tile([128, TT, HID], f8, name="vtok")
        qT = res.tile([128, S], dt.bfloat16, name="qT")
        kT = res.tile([128, S], dt.bfloat16, name="kT")

        # constant ones-row (partition 0) as the 5th contraction slice
        nc.vector.memset(normedT[:, KD, :], 0.0)
        nc.vector.memset(normedT[0:1, KD, :], 1.0)

        def body():
            # ---------------- Phase 1: LN + transpose ----------------
            with tc.tile_pool(name="p1s", bufs=3) as p1s, \
                 tc.tile_pool(name="p1st", bufs=4) as p1st, \
                 tc.tile_pool(name="p1p", bufs=4, space="PSUM") as p1p:
                for t in range(TT):
                    tsl = slice(t * 128, (t + 1) * 128)
                    nc.sync.dma_start(xt[:, t, :], ap_x[tsl, :])
                    bns = p1st.tile([128, 6], dt.float32, name="bns", tag="bns")
                    nc.vector.bn_stats(bns[:], xt[:, t, :])
                    bna = p1st.tile([128, 2], dt.float32, name="bna", tag="bna")
                    nc.vector.bn_aggr(bna[:], bns[:])
                    std = p1st.tile([128, 1], dt.float32, name="std", tag="std")
                    nc.scalar.activation(std[:], bna[:, 1:2], AF.Sqrt,
                                         bias=eps_t[:], scale=1.0)
                    rstd = p1st.tile([128, 1], dt.float32, name="rstd", tag="rstd")
                    nc.vector.reciprocal(rstd[:], std[:])
                    nmur = p1st.tile([128, 1], dt.float32, name="nmur", tag="nmur")
                    nc.vector.tensor_scalar(nmur[:], bna[:, 0:1], rstd[:], -1.0,
                                            op0=ALU.mult, op1=ALU.mult)
                    nb = p1s.tile([128, D], dt.bfloat16, name="nb", tag="nb")
                    nc.scalar.activation(nb[:], xt[:, t, :], AF.Identity,
                                         bias=nmur[:], scale=rstd[:])
                    for k in range(KD):
                        trp = p1p.tile([128, 128], dt.bfloat16, name="trp", tag="trp")
                        nc.tensor.transpose(trp[:], nb[:, k * 128:(k + 1) * 128], idb[:])
                        nc.scalar.copy(normedT[:, k, tsl], trp[:])

            # ---------------- Phase 2: ZT/qT/kT and v ----------------
            with tc.tile_pool(name="p2s", bufs=2) as p2s, \
                 tc.tile_pool(name="p2zp", bufs=2, space="PSUM") as p2zp, \
                 tc.tile_pool(name="p2vp", bufs=4, space="PSUM") as p2vp:
                for n in range(4):
                    nsl = slice(n * 512, (n + 1) * 512)
                    zp = p2zp.tile([128, 512], dt.float32, name="zp", tag="zp")
                    for p in range(2):
                        nc.tensor.matmul(zp[:], wqk[:, 2 * p:2 * p + 2, :],
                                         normedT[:, 2 * p:2 * p + 2, nsl],
                                         start=(p == 0), stop=(p == 1), perf_mode=DR)
                    zt = p2s.tile([128, 512], dt.float32, name="zt", tag="zt")
                    nc.scalar.activation(zt[:], zp[:], AF.Silu,
                                         bias=vecs["bqk"][:], scale=SC["inv_wqk"])
                    nc.vector.tensor_scalar(qT[:, nsl], zt[:], vecs["gq"][:],
                                            vecs["bq"][:], op0=ALU.mult, op1=ALU.add)
                    nc.vector.tensor_scalar(kT[:, nsl], zt[:], vecs["gk"][:],
                                            vecs["bk"][:], op0=ALU.mult, op1=ALU.add)

                for t in range(TT):
                    tsl = slice(t * 128, (t + 1) * 128)
                    for n in range(2):
                        nsl = slice(n * 512, (n + 1) * 512)
                        vp = p2vp.tile([128, 512], dt.float32, name="vp", tag="vp")
                        for p in range(2):
                            nc.tensor.matmul(vp[:], normedT[:, 2 * p:2 * p + 2, tsl],
                                             whv[:, 2 * p:2 * p + 2, nsl],
                                             start=(p == 0), stop=False, perf_mode=DR)
                        nc.tensor.matmul(vp[:], normedT[:, KD:KD + 1, tsl],
                                         whv[:, KD:KD + 1, nsl],
                                         start=False, stop=True)
                        nc.scalar.activation(vtok[:, t, nsl], vp[:], AF.Silu,
                                             bias=0.0, scale=SC["inv_wh"])

            # ---------------- Phase 3: gate + attention + output ----------------
            with tc.tile_pool(name="p3s", bufs=3) as p3s, \
                 tc.tile_pool(name="p3b", bufs=2) as p3b, \
                 tc.tile_pool(name="p3o", bufs=3) as p3o, \
                 tc.tile_pool(name="gp_p", bufs=2, space="PSUM") as gp_p, \
                 tc.tile_pool(name="sp_p", bufs=2, space="PSUM") as sp_p, \
                 tc.tile_pool(name="vt_p", bufs=2, space="PSUM") as vt_p, \
                 tc.tile_pool(name="ot_p", bufs=2, space="PSUM") as ot_p:
                sq_idx = 0
                for qc in range(NQ):
                    qsl = slice(qc * 512, (qc + 1) * 512)
                    # gate chunk, feature-major f32
                    gates = p3b.tile([128, MH, 512], dt.float32, name="gates", tag="gates")
                    for m in range(MH):
                        gp = gp_p.tile([128, 512], dt.float32, name="gp", tag="gp")
                        for p in range(2):
                            nc.tensor.matmul(gp[:], whg[:, 2 * p:2 * p + 2, m * 128:(m + 1) * 128],
                                             normedT[:, 2 * p:2 * p + 2, qsl],
                                             start=(p == 0), stop=(p == 1), perf_mode=DR)
                        nc.scalar.activation(gates[:, m, :], gp[:], AF.Silu,
                                             bias=vecs["bhg"][:, m:m + 1],
                                             scale=SC["inv_wh"])
                    # sim' -> at (relu^2, scaled fp8)
                    ats = p3b.tile([128, TT, 512], f8, name="ats", tag="ats")
                    for kt in range(TT):
                        sp = sp_p.tile([128, 512], dt.float32, name="sp", tag="sp")
                        nc.tensor.matmul(sp[:], kT[:, kt * 128:(kt + 1) * 128],
                                         qT[:, qsl], start=True, stop=True)
                        rl = p3s.tile([128, 512], dt.float32, name="rl", tag="rl")
                        nc.scalar.activation(rl[:], sp[:], AF.Relu, bias=0.0, scale=1.0)
                        if sq_idx < N_ACT_SQ:
                            nc.scalar.activation(ats[:, kt, :], rl[:], AF.Square,
                                                 bias=0.0, scale=1.0)
                        else:
                            nc.vector.tensor_tensor(ats[:, kt, :], rl[:], rl[:],
                                                    op=ALU.mult)
                        sq_idx += 1
                    # VT accumulate + gating
                    vtgs = p3b.tile([128, MH, 512], f8, name="vtgs", tag="vtgs")
                    for m in range(MH):
                        vt = vt_p.tile([128, 512], dt.float32, name="vt", tag="vt")
                        for p in range(TT // 2):
                            nc.tensor.matmul(vt[:], vtok[:, 2 * p:2 * p + 2, m * 128:(m + 1) * 128],
                                             ats[:, 2 * p:2 * p + 2, :],
                                             start=(p == 0), stop=(p == TT // 2 - 1),
                                             perf_mode=DR)
                        nc.vector.scalar_tensor_tensor(vtgs[:, m, :], vt[:], SC["s_vtg"],
                                                       gates[:, m, :],
                                                       op0=ALU.mult, op1=ALU.mult)
                    # output projection, token-major; residual + bias on DVE
                    for tt in range(4):
                        t = qc * 4 + tt
                        op = ot_p.tile([128, 512], dt.float32, name="op", tag="op")
                        for p in range(MH // 2):
                            nc.tensor.matmul(op[:], vtgs[:, 2 * p:2 * p + 2, tt * 128:(tt + 1) * 128],
                                             wo[:, 2 * p:2 * p + 2, :],
                                             start=(p == 0), stop=(p == MH // 2 - 1),
                                             perf_mode=DR)
                        tmp = p3o.tile([128, D], dt.float32, name="tmp", tag="tmp")
                        nc.vector.scalar_tensor_tensor(tmp[:], op[:], SC["s_fin"],
                                                       vecs["bo_row"][:],
                                                       op0=ALU.mult, op1=ALU.add)
                        ot = p3o.tile([128, D], dt.float32, name="ot", tag="ot")
                        nc.vector.tensor_tensor(ot[:], tmp[:], xt[:, t, :], op=ALU.add)
                        nc.sync.dma_start(ap_out[t * 128:(t + 1) * 128, :], ot[:])

        if loops == 1:
            body()
        else:
            with tc.For_i(0, loops, 1):
                body()


# scale constants used at trace time; set by _prep_maps before _build
SC = {"inv_wh": 1.0, "inv_wqk": 1.0, "s_vtg": 1.0, "s_fin": 1.0}


def _silu(z):
    return z / (1.0 + np.exp(-z))


def _pow2(v, lo=-60, hi=60):
    return float(2.0 ** int(np.clip(np.floor(np.log2(max(v, 1e-300))), lo, hi)))


def _calibrate(x, ln_g, ln_b, Wh_eff, bh_eff, Wqk_eff, bqk_eff, gamma, beta, Wo):
    """Pick power-of-2 fp8 scales from a 64-token sample (host-side)."""
    xs = np.asarray(x[0, ::32, :], np.float64)  # [64, D]
    mu = xs.mean(-1, keepdims=True)
    sd = np.sqrt(((xs - mu) ** 2).mean(-1, keepdims=True) + EPS)
    ns = (xs - mu) / sd  # ln_g/ln_b already folded into *_eff
    Zs = _silu(ns @ Wqk_eff + bqk_eff)           # [64, QK]
    qs = Zs * gamma[0] + beta[0]
    ks = Zs * gamma[1] + beta[1]
    sim_s = (qs @ ks.T) / S
    m_sim = float(np.abs(sim_s).max()) + 1e-300
    s_sim = _pow2(1.5 / m_sim)                   # |sim'| <~ 1.5, at' <~ 2.3 (<<240)
    a = _pow2(np.sqrt(s_sim))
    b = s_sim / a

    vs = _silu(ns @ Wh_eff[:, :HID] + bh_eff[:HID])
    gs = _silu(ns @ Wh_eff[:, HID:] + bh_eff[HID:])
    at_s = np.square(np.maximum(sim_s * s_sim, 0.0))
    # A is nonnegative, so A@v has a coherent component along per-column
    # means of v on top of the random-walk part.
    vbar = float(np.abs(vs.mean(0)).max())
    vp_est = (S * at_s.mean() * vbar
              + 3.0 * np.sqrt(S * np.mean(at_s ** 2)) * (np.std(vs) + 1e-30)
              + 1e-300)
    vtg_est = vp_est * (np.abs(gs).max() + 1e-30)
    s_vtg = _pow2(4.0 / vtg_est, lo=-40, hi=40)  # |vtg| target ~4, ~60x margin
    return s_sim, a, b, s_vtg


def _prep_maps(inputs):
    x = np.asarray(inputs["x"], np.float32)
    ln_g = np.asarray(inputs["ln_g"], np.float64)
    ln_b = np.asarray(inputs["ln_b"], np.float64)
    Wh = np.asarray(inputs["Wh"], np.float64)
    bh = np.asarray(inputs["bh"], np.float64)
    Wqk = np.asarray(inputs["Wqk"], np.float64)
    bqk = np.asarray(inputs["bqk"], np.float64)
    gamma = np.asarray(inputs["gamma"], np.float64)
    beta = np.asarray(inputs["beta"], np.float64)
    Wo = np.asarray(inputs["Wo"], np.float64)
    bo = np.asarray(inputs["bo"], np.float64)

    Wh_eff = ln_g[:, None] * Wh
    bh_eff = bh + ln_b @ Wh
    Wqk_eff = ln_g[:, None] * Wqk
    bqk_eff = bqk + ln_b @ Wqk

    s_wh = _pow2(128.0 / (np.abs(Wh_eff).max() + np.abs(bh_eff).max() + 1e-30))
    s_wqk = _pow2(128.0 / (np.abs(Wqk_eff).max() + 1e-30))
    s_wo = _pow2(128.0 / (np.abs(Wo).max() + 1e-30))
    s_sim, a, b, s_vtg = _calibrate(x, ln_g, ln_b, Wh_eff, bh_eff, Wqk_eff,
                                    bqk_eff, gamma, beta, Wo)

    SC["inv_wh"] = 1.0 / s_wh
    SC["inv_wqk"] = 1.0 / s_wqk
    SC["s_vtg"] = s_vtg
    SC["s_fin"] = 1.0 / (s_sim * s_sim * s_vtg * s_wo)

    def to8(w):
        return np.clip(w, -240.0, 240.0).astype(np.float32).astype(F8)

    Whv = Wh_eff[:, :HID] * s_wh
    Whg = Wh_eff[:, HID:] * s_wh
    bhv = bh_eff[:HID] * s_wh
    bhg = bh_eff[HID:]

    whv_np = np.zeros((128, KD + 1, HID), np.float32)
    for k in range(KD):
        whv_np[:, k, :] = Whv[k * 128:(k + 1) * 128, :]
    whv_np[0, KD, :] = bhv
    whg_np = np.stack([Whg[k * 128:(k + 1) * 128, :] for k in range(KD)], axis=1)
    wqk_np = np.stack([(Wqk_eff * s_wqk)[k * 128:(k + 1) * 128, :] for k in range(KD)], axis=1)
    wo_np = np.stack([(Wo * s_wo)[k * 128:(k + 1) * 128, :] for k in range(MH)], axis=1)

    common = {
        "whv": to8(whv_np),
        "whg": to8(whg_np.astype(np.float32)),
        "wqk": to8(wqk_np.astype(np.float32)),
        "wo": to8(wo_np.astype(np.float32)),
        "bhg": np.ascontiguousarray(bhg.reshape(MH, 128).T).astype(np.float32),
        "bqk": bqk_eff.reshape(128, 1).astype(np.float32),
        "gq": (gamma[0] * a).reshape(128, 1).astype(np.float32),
        "bq": (beta[0] * a).reshape(128, 1).astype(np.float32),
        "gk": (gamma[1] * b / S).reshape(128, 1).astype(np.float32),
        "bk": (beta[1] * b / S).reshape(128, 1).astype(np.float32),
        "bo_row": np.ascontiguousarray(np.broadcast_to(bo, (128, D))).astype(np.float32),
        "idb": np.eye(128, dtype=np.float32).astype(BF16),
    }
    return [{**common, "x": np.ascontiguousarray(x[bb])} for bb in range(B)]


def kernel(**inputs):
    in_maps = _prep_maps(inputs)
    key = (SC["inv_wh"], SC["inv_wqk"], SC["s_vtg"], SC["s_fin"])
    if _COMPILED.get("key") != key:
        _COMPILED["nc"] = _build(loops=1)
        _COMPILED["key"] = key
    nc = _COMPILED["nc"]
    res = run_bass_kernel_spmd(nc, in_maps, core_ids=list(range(NCORES)))
    out = np.stack([res.results[c]["out"] for c in range(B)], axis=0)
    return out.astype(np.float32)
